# revision 76
# baseline (speedup 1.0000x reference)
"""Trainium2 Bass kernel for CrossAttention1D.

Strategy: data-parallel over batch B=8 (one batch per NeuronCore).
Per core, everything is fused in one program, all matmuls in f16
(fp32 PSUM accumulation):
  - xq/xk/xv arrive host-pretransposed to dim-major f16 layout, loaded
    with plain contiguous DMAs (no PE transposes, no PSUM round-trip)
  - Q^T/K^T (dim-major) projections with the bias-add fused into the
    mandatory PSUM->SBUF copy on the vector engine
  - V is projected token-major into a padded [tok, H*65] layout whose
    65th column per head is 1.0, so a single M=65 PV matmul produces
    both the attention output and the softmax denominator (no separate
    ones-matmul)
  - attention runs as a flat 128-slot software pipeline: slot g issues
    QK(g+1) scores, then PV(g-1) — PV lags its exp by a full slot so
    the exp->PV semaphore latency is never exposed; attn_avg = (Q K^T)
    * SCALE / H (sum of per-head dot products == full 1024-dim dot
    product) is spread one matmul per slot as exp-independent filler
  - per head: scores computed TRANSPOSED [k_tok, q_tok] so softmax'd
    probabilities feed PV directly; exp on ScalarE from PSUM; PV output
    is staged out of PSUM immediately and normalized one head deferred
    (reciprocal on DVE, broadcast/multiply on the idle Pool engine)
  - final projection from OT (already in lhsT layout)
SCALE is folded into Wq/bq on the host; outputs are f16, upcast on host.
A short burst of throwaway matmuls keeps the PE p-state clock warm
through the startup DMA wait.
"""

import sys

if "/opt/trn_rl_repo" not in sys.path:
    sys.path.insert(0, "/opt/trn_rl_repo")

import numpy as np

import concourse.bacc as bacc
import concourse.mybir as mybir
from concourse.bass_utils import run_bass_kernel_spmd
from concourse.tile import TileContext

F32 = mybir.dt.float32
F16 = mybir.dt.float16
AF = mybir.ActivationFunctionType

N = 1024   # tokens
C = 1024   # model dim
H = 16     # heads
D = 64     # head dim
DV1 = D + 1  # head dim + fused ones column
P = 128    # partitions
NT = N // P   # 8 token tiles
CT = C // P   # 8 contraction tiles
NCH = 2       # 512-wide chunks per 1024
SCALE = D ** -0.5
B = 8


def _emit(nc, reps=1):
    # xq/xk/xv arrive host-pretransposed to dim-major [2*P, CT*512]: row
    # (half*P + p), col (ct*512 + t) = x[half*512 + t, ct*128 + p].  Each
    # half is one fully-contiguous DMA (8KB runs) — cheaper than the xbar
    # DMA-transpose path and one less instruction class on hardware.
    xq = nc.dram_tensor("xq", [2 * P, CT * 512], F16, kind="ExternalInput")
    xk = nc.dram_tensor("xk", [2 * P, CT * 512], F16, kind="ExternalInput")
    xv = nc.dram_tensor("xv", [2 * P, CT * 512], F16, kind="ExternalInput")
    Wq = nc.dram_tensor("Wq", [C, C], F16, kind="ExternalInput")
    Wk = nc.dram_tensor("Wk", [C, C], F16, kind="ExternalInput")
    Wv = nc.dram_tensor("Wv", [C, C], F16, kind="ExternalInput")
    Wp = nc.dram_tensor("Wp", [C, C], F16, kind="ExternalInput")
    # bq/bk arrive host-pretransposed to [P, CT] (partition-major) so a single
    # contiguous DMA lands them in the per-partition layout
    bq = nc.dram_tensor("bq", [P, CT], F32, kind="ExternalInput")
    bk = nc.dram_tensor("bk", [P, CT], F32, kind="ExternalInput")
    bv = nc.dram_tensor("bv", [C], F32, kind="ExternalInput")
    bp = nc.dram_tensor("bp", [C], F32, kind="ExternalInput")
    out = nc.dram_tensor("out", [N, C], F16, kind="ExternalOutput")
    attn_avg = nc.dram_tensor("attn_avg", [N, N], F16, kind="ExternalOutput")

    with TileContext(nc) as tc:
      for _rep in range(reps):
        with (
            tc.tile_pool(name="persist", bufs=1) as persist,
            tc.tile_pool(name="ps", bufs=2, space="PSUM") as ps,
            tc.tile_pool(name="pa", bufs=1, space="PSUM") as pap,
            tc.tile_pool(name="po", bufs=1, space="PSUM") as po,
        ):
            bq_t = persist.tile([P, CT], F32, tag="bq")
            bk_t = persist.tile([P, CT], F32, tag="bk")
            bv_bc = persist.tile([P, C], F32, tag="bv")
            bp_bc = persist.tile([P, C], F32, tag="bp")

            # persistent activations
            QT = persist.tile([P, CT, N], F16, tag="QT")      # Q^T: [dim, tok]
            KT = persist.tile([P, CT, N], F16, tag="KT")      # K^T: [dim, tok]
            VE = persist.tile([P, NT, H * DV1], F16, tag="VE")  # V: [tok, H*(D+1)]
            OT = persist.tile([P, CT, N], F16, tag="OT")      # out^T: [dim, tok]

            # ones column per head in VE (feeds the fused denominator row)
            for h in range(H):
                nc.vector.memset(VE[:, :, h * DV1 + D : (h + 1) * DV1], 1.0)

            # PE p-state warm-up: the tensor engine clock ramps only after
            # ~3us of continuous work, and resets on idle.  Burn the startup
            # DMA wait with throwaway matmuls on zeroed tiles so the real
            # pipeline starts at full clock.
            zsa = persist.tile([P, P], F16, tag="zsa")
            zsb = persist.tile([P, 512], F16, tag="zsb")
            nc.vector.memset(zsa[:], 0.0)
            nc.vector.memset(zsb[:], 0.0)
            warm_ps = pap.tile([P, N], F32, tag="pa")
            for _ in range(12):
                nc.tensor.matmul(warm_ps[:, 0:512], zsa[:], zsb[:], start=True, stop=True)

            # ================= phase 1: transposed loads + projections ========
            with (
                tc.tile_pool(name="xT", bufs=4) as xTp,
                tc.tile_pool(name="wfull", bufs=3) as wfullp,
                tc.tile_pool(name="outsb", bufs=3) as outsb,
                tc.tile_pool(name="pT", bufs=4) as pTp,
                tc.tile_pool(name="misc", bufs=2) as misc,
                tc.tile_pool(name="ogp", bufs=2) as ogp,
            ):
                def transpose_half(x_dram, half):
                    # separate tile per half => precise dependencies, so the
                    # first projection group starts as soon as half 0 lands
                    xTh = xTp.tile([P, CT, 512], F16, tag="xT")
                    nc.sync.dma_start(
                        out=xTh[:],
                        in_=x_dram[half * P : (half + 1) * P, :].rearrange(
                            "p (ct t) -> p ct t", t=512
                        ),
                    )
                    return xTh

                def load_w_full(W_dram):
                    # whole weight in one DMA: [P, ct, m] layout, 2KB runs
                    w_t = wfullp.tile([P, CT, C], F16, tag="w")
                    nc.sync.dma_start(
                        out=w_t[:], in_=W_dram[:, :].rearrange("(ct p) m -> p ct m", p=P)
                    )
                    return w_t

                def proj_dim_major(lhs_sel, xTh, bias_t, dst):
                    # dst[:, mt, :] = (W^T x^T)[dim-tile mt, tok] + bias
                    for mt in range(CT):
                        pp = ps.tile([P, N], F32, tag="ps")
                        for nch in range(NCH):
                            for ct in range(CT):
                                nc.tensor.matmul(
                                    pp[:, nch * 512 : (nch + 1) * 512],
                                    lhs_sel(mt, ct),
                                    xTh[nch][:, ct, :],
                                    start=(ct == 0),
                                    stop=(ct == CT - 1),
                                )
                        nc.vector.tensor_scalar_add(
                            dst[:, mt, :], pp[:], bias_t[:, mt : mt + 1]
                        )

                # startup order: Q weights (first half, own tile for a precise
                # dependency), xq half 0 => first matmul group fires as soon
                # as both land; rest pipelines
                def load_wq_half(hf):
                    wqh = wfullp.tile([P, CT, 512], F16, tag="w")
                    nc.sync.dma_start(
                        out=wqh[:],
                        in_=Wq[:, hf * 512 : (hf + 1) * 512].rearrange(
                            "(ct p) m -> p ct m", p=P
                        ),
                    )
                    return wqh

                wq_h = [load_wq_half(0)]
                xq0 = transpose_half(xq, 0)
                xq1 = transpose_half(xq, 1)
                wq_h.append(load_wq_half(1))
                # biases behind the startup transposes in the SP FIFO, so they
                # can't be hoisted ahead of them by an idle sequencer
                nc.sync.dma_start(out=bq_t[:], in_=bq[:, :])
                nc.sync.dma_start(out=bk_t[:], in_=bk[:, :])
                nc.sync.dma_start(out=bv_bc[0:1, :], in_=bv[None, :])
                nc.sync.dma_start(out=bp_bc[0:1, :], in_=bp[None, :])
                nc.gpsimd.partition_broadcast(bv_bc[:], bv_bc[0:1, :], channels=P)
                nc.gpsimd.partition_broadcast(bp_bc[:], bp_bc[0:1, :], channels=P)
                proj_dim_major(
                    lambda mt, ct: wq_h[mt // 4][:, ct, (mt % 4) * P : (mt % 4 + 1) * P],
                    [xq0, xq1],
                    bq_t,
                    QT,
                )

                xk0 = transpose_half(xk, 0)
                wk_t = load_w_full(Wk)
                xk1 = transpose_half(xk, 1)
                proj_dim_major(
                    lambda mt, ct: wk_t[:, ct, mt * P : (mt + 1) * P],
                    [xk0, xk1],
                    bk_t,
                    KT,
                )

                xv0 = transpose_half(xv, 0)
                xv1 = transpose_half(xv, 1)
                xvT = [xv0, xv1]
                wv_t = load_w_full(Wv)

                # V token-major into padded VE: VE[tt, h*65:(h*65+64)] =
                # xv @ Wv + bv
                def v_group(half, tt):
                    pv = ps.tile([P, 512], F32, tag="ps")
                    for ct in range(CT):
                        nc.tensor.matmul(
                            pv[:],
                            xvT[tt // 4][:, ct, (tt % 4) * P : (tt % 4 + 1) * P],
                            wv_t[:, ct, half * 512 : (half + 1) * 512],
                            start=(ct == 0),
                            stop=(ct == CT - 1),
                        )
                    dst = VE[
                        :, tt, half * 8 * DV1 : (half + 1) * 8 * DV1
                    ].rearrange("p (h x) -> p h x", x=DV1)[:, :, 0:D]
                    nc.vector.tensor_add(
                        dst,
                        pv[:].rearrange("p (h d) -> p h d", d=D),
                        bv_bc[:, half * 512 : (half + 1) * 512].rearrange(
                            "p (h d) -> p h d", d=D
                        ),
                    )

                for half in range(2):
                    for tt in range(NT):
                        v_group(half, tt)

                # final-projection weights: load early, transfer overlaps heads
                wp_t = load_w_full(Wp)

                # ============ phase 2: per-head attention + attn_avg ==========
                # attn_avg matmuls are spread one-per-kt-slot through the head
                # loop (16 slots per 2-head cycle = 16 accumulation steps of
                # one attn_avg row-tile), so the PE always has exp-independent
                # filler work and ScalarE stays saturated on exp
                pa = None
                ogs = []

                def normalize_head(hh, og):
                    ct_hh = hh // 2
                    po_hh = (hh % 2) * D
                    rc = misc.tile([1, N], F32, tag="rc")
                    nc.vector.reciprocal(rc[0:1, :], og[D : DV1, :])
                    bc = misc.tile([D, N], F32, tag="bc")
                    nc.gpsimd.partition_broadcast(bc[:], rc[0:1, :], channels=D)
                    nc.gpsimd.tensor_mul(OT[po_hh : po_hh + D, ct_hh, :], og[0:D, :], bc[:])

                def avg_step(qt, a):
                    # one accumulation step of attn_avg row-tile qt
                    nonlocal pa
                    ct_a, nch_a = a // 2, a % 2
                    if a == 0:
                        pa = pap.tile([P, N], F32, tag="pa")
                    nc.tensor.matmul(
                        pa[:, nch_a * 512 : (nch_a + 1) * 512],
                        QT[:, ct_a, qt * P : (qt + 1) * P],
                        KT[:, ct_a, nch_a * 512 : (nch_a + 1) * 512],
                        start=(ct_a == 0),
                        stop=(ct_a == CT - 1),
                    )

                NSLOT = H * NT  # 128 global attention slots

                def qk_slot(g):
                    # scores + exp for global slot g = (head, kt)
                    h, kt = g // NT, g % NT
                    ct_h, po_h = h // 2, (h % 2) * D
                    psum_s = ps.tile([P, N], F32, tag="ps")
                    for nch in range(NCH):
                        nc.tensor.matmul(
                            psum_s[:, nch * 512 : (nch + 1) * 512],
                            KT[po_h : po_h + D, ct_h, kt * P : (kt + 1) * P],
                            QT[po_h : po_h + D, ct_h, nch * 512 : (nch + 1) * 512],
                            start=True,
                            stop=True,
                        )
                    pt = pTp.tile([P, N], F16, tag="pT")
                    nc.scalar.activation(pt[:], psum_s[:], AF.Exp)
                    return pt

                # software pipeline: QK(g+1) issues at slot g, PV(g-1) runs at
                # slot g.  PV lags its exp by a full slot, so the exp->PV
                # semaphore latency is never exposed, and the og staging copy
                # has a whole slot of slack before the next head's first PV
                pts = {0: qk_slot(0)}
                psum_o = None
                for g in range(NSLOT + 1):
                    if g + 1 < NSLOT:
                        pts[g + 1] = qk_slot(g + 1)
                    gp = g - 1  # the slot whose PV work we do now
                    if gp < 0:
                        continue
                    h, kt = gp // NT, gp % NT
                    ct_h, po_h = h // 2, (h % 2) * D
                    qt = h // 2  # attn_avg tile accumulated during this cycle
                    if kt == 0:
                        # normalize the head-before-last here: the chain
                        # (recip -> Pool broadcast -> Pool mul) overlaps this
                        # head's compute and never gates PSUM reuse
                        if ogs and ogs[0][0] < h:
                            normalize_head(*ogs.pop(0))
                        psum_o = po.tile([DV1, N], F32, tag="po")
                    pt = pts.pop(gp)
                    for nch in range(NCH):
                        nc.tensor.matmul(
                            psum_o[:, nch * 512 : (nch + 1) * 512],
                            VE[:, kt, h * DV1 : (h + 1) * DV1],
                            pt[:, nch * 512 : (nch + 1) * 512],
                            start=(kt == 0),
                            stop=(kt == NT - 1),
                        )
                    # one attn_avg step per slot, shifted one slot late so the
                    # previous cycle's epilogue read never blocks the next
                    # cycle's first accumulation
                    t = gp % (2 * NT)  # 0..15 within the 2-head cycle
                    if t >= 1:
                        avg_step(qt, t - 1)
                    if kt < NT - 1:
                        continue
                    # ---- end of head h ----
                    if h < H - 1:
                        # stage PV output out of PSUM immediately so the next
                        # head's accumulation isn't gated on the normalize
                        # chain; split in halves so the first half of the PSUM
                        # bank frees one copy earlier
                        og = ogp.tile([DV1, N], F32, tag="og")
                        nc.vector.tensor_copy(og[:], psum_o[:])
                        ogs.append((h, og))
                    else:
                        # last head: normalize straight from PSUM, split in
                        # halves, so phase 3's ct=7 dependency clears early
                        rc = misc.tile([1, N], F32, tag="rc")
                        nc.vector.reciprocal(rc[0:1, :], psum_o[D : DV1, :])
                        bc = misc.tile([D, N], F32, tag="bc")
                        for hf in range(2):
                            sl = slice(hf * 512, (hf + 1) * 512)
                            nc.gpsimd.partition_broadcast(bc[:, sl], rc[0:1, sl], channels=D)
                            nc.vector.tensor_mul(
                                OT[po_h : po_h + D, ct_h, sl], psum_o[0:D, sl], bc[:, sl]
                            )
                    if h % 2 == 1:
                        avg_step(qt, 15)
                        av = outsb.tile([P, N], F16, tag="o")
                        nc.vector.tensor_scalar_mul(av[:], pa[:], 1.0 / H)
                        nc.sync.dma_start(out=attn_avg[qt * P : (qt + 1) * P, :], in_=av[:])

                # ================= phase 3: final projection ==================
                for qt in range(NT):
                    ot = outsb.tile([P, C], F16, tag="o")
                    if qt < NT - 1:
                        pf = ps.tile([P, N], F32, tag="ps")
                        for ct in range(CT):
                            for nch in range(NCH):
                                nc.tensor.matmul(
                                    pf[:, nch * 512 : (nch + 1) * 512],
                                    OT[:, ct, qt * P : (qt + 1) * P],
                                    wp_t[:, ct, nch * 512 : (nch + 1) * 512],
                                    start=(ct == 0),
                                    stop=(ct == CT - 1),
                                )
                        nc.vector.tensor_add(ot[:], pf[:], bp_bc[:])
                        nc.sync.dma_start(out=out[qt * P : (qt + 1) * P, :], in_=ot[:])
                    else:
                        # last tile: nch-major with separate psum tiles per
                        # half, so each half's epilogue + DMA overlaps the
                        # other half's matmuls and the drain tail shortens
                        for nch in range(NCH):
                            sl = slice(nch * 512, (nch + 1) * 512)
                            pfh = ps.tile([P, 512], F32, tag="ps")
                            for ct in range(CT):
                                nc.tensor.matmul(
                                    pfh[:],
                                    OT[:, ct, qt * P : (qt + 1) * P],
                                    wp_t[:, ct, sl],
                                    start=(ct == 0),
                                    stop=(ct == CT - 1),
                                )
                            nc.vector.tensor_add(ot[:, sl], pfh[:], bp_bc[:, sl])
                            nc.sync.dma_start(
                                out=out[qt * P : (qt + 1) * P, sl], in_=ot[:, sl]
                            )

    return nc


LAST_RESULT = None
_NC_CACHE = {}


def _get_nc(reps=1):
    if reps not in _NC_CACHE:
        nc = bacc.Bacc("TRN2", target_bir_lowering=False, debug=False)
        _emit(nc, reps)
        nc.compile()
        _NC_CACHE[reps] = nc
    return _NC_CACHE[reps]


def kernel(xq, xk, xv, Wq, bq, Wk, bk, Wv, bv, Wp, bp, **_ignored):
    nc = _get_nc()
    f16 = np.float16
    common = {
        "Wq": np.ascontiguousarray((np.asarray(Wq, np.float32) * np.float32(SCALE)).astype(f16)),
        "bq": np.ascontiguousarray(
            (np.asarray(bq, np.float32) * np.float32(SCALE)).reshape(CT, P).T
        ),
        "Wk": np.ascontiguousarray(np.asarray(Wk, np.float32).astype(f16)),
        "bk": np.ascontiguousarray(np.asarray(bk, np.float32).reshape(CT, P).T),
        "Wv": np.ascontiguousarray(np.asarray(Wv, np.float32).astype(f16)),
        "bv": np.ascontiguousarray(np.asarray(bv, np.float32)),
        "Wp": np.ascontiguousarray(np.asarray(Wp, np.float32).astype(f16)),
        "bp": np.ascontiguousarray(np.asarray(bp, np.float32)),
    }
    def prep_x(x):
        # host pre-transpose to the dim-major [2*P, CT*512] device layout:
        # row (half*P + p), col (ct*512 + t) = x[half*512 + t, ct*128 + p]
        full = np.asarray(x).astype(f16).T.reshape(CT, P, N).transpose(1, 0, 2)
        return np.ascontiguousarray(
            np.concatenate(
                [full[:, :, 0:512].reshape(P, -1), full[:, :, 512:].reshape(P, -1)],
                axis=0,
            )
        )

    in_maps = []
    for b in range(B):
        m = dict(common)
        m["xq"] = prep_x(xq[b])
        m["xk"] = prep_x(xk[b])
        m["xv"] = prep_x(xv[b])
        in_maps.append(m)
    res = run_bass_kernel_spmd(nc, in_maps, list(range(B)))
    global LAST_RESULT
    LAST_RESULT = res
    out = np.stack([np.asarray(res.results[b]["out"], np.float32) for b in range(B)])
    attn_avg = np.stack(
        [np.asarray(res.results[b]["attn_avg"], np.float32) for b in range(B)]
    )
    return out, attn_avg


# revision 79
# speedup vs baseline: 1.0008x; 1.0008x over previous
"""Trainium2 Bass kernel for CrossAttention1D.

Strategy: data-parallel over batch B=8 (one batch per NeuronCore).
Per core, everything is fused in one program, all matmuls in f16
(fp32 PSUM accumulation):
  - xq/xk/xv arrive host-pretransposed to dim-major f16 layout, loaded
    with plain contiguous DMAs (no PE transposes, no PSUM round-trip)
  - Q^T/K^T (dim-major) projections with the bias-add fused into the
    mandatory PSUM->SBUF copy on the vector engine
  - V is projected token-major into a padded [tok, H*65] layout whose
    65th column per head is 1.0, so a single M=65 PV matmul produces
    both the attention output and the softmax denominator (no separate
    ones-matmul)
  - attention runs as a flat 128-slot software pipeline: slot g issues
    QK(g+1) scores, then PV(g-1) — PV lags its exp by a full slot so
    the exp->PV semaphore latency is never exposed; attn_avg = (Q K^T)
    * SCALE / H (sum of per-head dot products == full 1024-dim dot
    product) is spread one matmul per slot as exp-independent filler
  - per head: scores computed TRANSPOSED [k_tok, q_tok] so softmax'd
    probabilities feed PV directly; exp on ScalarE from PSUM; PV output
    is staged out of PSUM immediately and normalized one head deferred
    (reciprocal on DVE, broadcast/multiply on the idle Pool engine)
  - final projection from OT (already in lhsT layout)
SCALE is folded into Wq/bq on the host; outputs are f16, upcast on host.
A short burst of throwaway matmuls keeps the PE p-state clock warm
through the startup DMA wait.
"""

import sys

if "/opt/trn_rl_repo" not in sys.path:
    sys.path.insert(0, "/opt/trn_rl_repo")

import numpy as np

import concourse.bacc as bacc
import concourse.mybir as mybir
from concourse.bass_utils import run_bass_kernel_spmd
from concourse.tile import TileContext

F32 = mybir.dt.float32
F16 = mybir.dt.float16
AF = mybir.ActivationFunctionType

N = 1024   # tokens
C = 1024   # model dim
H = 16     # heads
D = 64     # head dim
DV1 = D + 1  # head dim + fused ones column
P = 128    # partitions
NT = N // P   # 8 token tiles
CT = C // P   # 8 contraction tiles
NCH = 2       # 512-wide chunks per 1024
SCALE = D ** -0.5
B = 8


def _emit(nc, reps=1):
    # xq/xk/xv arrive host-pretransposed to dim-major [2*P, CT*512]: row
    # (half*P + p), col (ct*512 + t) = x[half*512 + t, ct*128 + p].  Each
    # half is one fully-contiguous DMA (8KB runs) — cheaper than the xbar
    # DMA-transpose path and one less instruction class on hardware.
    xq = nc.dram_tensor("xq", [2 * P, CT * 512], F16, kind="ExternalInput")
    xk = nc.dram_tensor("xk", [2 * P, CT * 512], F16, kind="ExternalInput")
    xv = nc.dram_tensor("xv", [2 * P, CT * 512], F16, kind="ExternalInput")
    Wq = nc.dram_tensor("Wq", [C, C], F16, kind="ExternalInput")
    Wk = nc.dram_tensor("Wk", [C, C], F16, kind="ExternalInput")
    Wv = nc.dram_tensor("Wv", [C, C], F16, kind="ExternalInput")
    Wp = nc.dram_tensor("Wp", [C, C], F16, kind="ExternalInput")
    # bq/bk arrive host-pretransposed to [P, CT] (partition-major) so a single
    # contiguous DMA lands them in the per-partition layout
    bq = nc.dram_tensor("bq", [P, CT], F32, kind="ExternalInput")
    bk = nc.dram_tensor("bk", [P, CT], F32, kind="ExternalInput")
    bv = nc.dram_tensor("bv", [C], F32, kind="ExternalInput")
    bp = nc.dram_tensor("bp", [C], F32, kind="ExternalInput")
    out = nc.dram_tensor("out", [N, C], F16, kind="ExternalOutput")
    attn_avg = nc.dram_tensor("attn_avg", [N, N], F16, kind="ExternalOutput")

    with TileContext(nc) as tc:
      for _rep in range(reps):
        with (
            tc.tile_pool(name="persist", bufs=1) as persist,
            tc.tile_pool(name="ps", bufs=2, space="PSUM") as ps,
            tc.tile_pool(name="pa", bufs=1, space="PSUM") as pap,
            tc.tile_pool(name="po", bufs=1, space="PSUM") as po,
        ):
            bq_t = persist.tile([P, CT], F32, tag="bq")
            bk_t = persist.tile([P, CT], F32, tag="bk")
            bv_bc = persist.tile([P, C], F32, tag="bv")
            bp_bc = persist.tile([P, C], F32, tag="bp")

            # persistent activations
            QT = persist.tile([P, CT, N], F16, tag="QT")      # Q^T: [dim, tok]
            KT = persist.tile([P, CT, N], F16, tag="KT")      # K^T: [dim, tok]
            VE = persist.tile([P, NT, H * DV1], F16, tag="VE")  # V: [tok, H*(D+1)]
            OT = persist.tile([P, CT, N], F16, tag="OT")      # out^T: [dim, tok]

            # ones column per head in VE (feeds the fused denominator row)
            for h in range(H):
                nc.vector.memset(VE[:, :, h * DV1 + D : (h + 1) * DV1], 1.0)

            # PE p-state warm-up: the tensor engine clock ramps only after
            # ~3us of continuous work, and resets on idle.  Burn the startup
            # DMA wait with throwaway matmuls on zeroed tiles so the real
            # pipeline starts at full clock.
            zsa = persist.tile([P, P], F16, tag="zsa")
            zsb = persist.tile([P, 512], F16, tag="zsb")
            nc.vector.memset(zsa[:], 0.0)
            nc.vector.memset(zsb[:], 0.0)
            warm_ps = pap.tile([P, N], F32, tag="pa")
            for _ in range(12):
                nc.tensor.matmul(warm_ps[:, 0:512], zsa[:], zsb[:], start=True, stop=True)

            # ================= phase 1: transposed loads + projections ========
            with (
                tc.tile_pool(name="xT", bufs=4) as xTp,
                tc.tile_pool(name="wfull", bufs=3) as wfullp,
                tc.tile_pool(name="outsb", bufs=3) as outsb,
                tc.tile_pool(name="pT", bufs=4) as pTp,
                tc.tile_pool(name="misc", bufs=2) as misc,
                tc.tile_pool(name="ogp", bufs=2) as ogp,
            ):
                def transpose_half(x_dram, half):
                    # separate tile per half => precise dependencies, so the
                    # first projection group starts as soon as half 0 lands
                    xTh = xTp.tile([P, CT, 512], F16, tag="xT")
                    nc.sync.dma_start(
                        out=xTh[:],
                        in_=x_dram[half * P : (half + 1) * P, :].rearrange(
                            "p (ct t) -> p ct t", t=512
                        ),
                    )
                    return xTh

                def load_w_full(W_dram):
                    # whole weight in one DMA: [P, ct, m] layout, 2KB runs
                    w_t = wfullp.tile([P, CT, C], F16, tag="w")
                    nc.sync.dma_start(
                        out=w_t[:], in_=W_dram[:, :].rearrange("(ct p) m -> p ct m", p=P)
                    )
                    return w_t

                def proj_dim_major(lhs_sel, xTh, bias_t, dst):
                    # dst[:, mt, :] = (W^T x^T)[dim-tile mt, tok] + bias
                    for mt in range(CT):
                        pp = ps.tile([P, N], F32, tag="ps")
                        for nch in range(NCH):
                            for ct in range(CT):
                                nc.tensor.matmul(
                                    pp[:, nch * 512 : (nch + 1) * 512],
                                    lhs_sel(mt, ct),
                                    xTh[nch][:, ct, :],
                                    start=(ct == 0),
                                    stop=(ct == CT - 1),
                                )
                        nc.vector.tensor_scalar_add(
                            dst[:, mt, :], pp[:], bias_t[:, mt : mt + 1]
                        )

                # startup order: Q weights (first half, own tile for a precise
                # dependency), xq half 0 => first matmul group fires as soon
                # as both land; rest pipelines
                def load_wq_half(hf):
                    wqh = wfullp.tile([P, CT, 512], F16, tag="w")
                    nc.sync.dma_start(
                        out=wqh[:],
                        in_=Wq[:, hf * 512 : (hf + 1) * 512].rearrange(
                            "(ct p) m -> p ct m", p=P
                        ),
                    )
                    return wqh

                wq_h = [load_wq_half(0)]
                xq0 = transpose_half(xq, 0)
                xq1 = transpose_half(xq, 1)
                wq_h.append(load_wq_half(1))
                # biases behind the startup transposes in the SP FIFO, so they
                # can't be hoisted ahead of them by an idle sequencer
                nc.sync.dma_start(out=bq_t[:], in_=bq[:, :])
                nc.sync.dma_start(out=bk_t[:], in_=bk[:, :])
                nc.sync.dma_start(out=bv_bc[0:1, :], in_=bv[None, :])
                nc.sync.dma_start(out=bp_bc[0:1, :], in_=bp[None, :])
                nc.gpsimd.partition_broadcast(bv_bc[:], bv_bc[0:1, :], channels=P)
                nc.gpsimd.partition_broadcast(bp_bc[:], bp_bc[0:1, :], channels=P)
                proj_dim_major(
                    lambda mt, ct: wq_h[mt // 4][:, ct, (mt % 4) * P : (mt % 4 + 1) * P],
                    [xq0, xq1],
                    bq_t,
                    QT,
                )

                xk0 = transpose_half(xk, 0)
                wk_t = load_w_full(Wk)
                xk1 = transpose_half(xk, 1)
                proj_dim_major(
                    lambda mt, ct: wk_t[:, ct, mt * P : (mt + 1) * P],
                    [xk0, xk1],
                    bk_t,
                    KT,
                )

                xv0 = transpose_half(xv, 0)
                xv1 = transpose_half(xv, 1)
                xvT = [xv0, xv1]
                wv_t = load_w_full(Wv)

                # V token-major into padded VE: VE[tt, h*65:(h*65+64)] =
                # xv @ Wv + bv
                def v_group(half, tt):
                    pv = ps.tile([P, 512], F32, tag="ps")
                    for ct in range(CT):
                        nc.tensor.matmul(
                            pv[:],
                            xvT[tt // 4][:, ct, (tt % 4) * P : (tt % 4 + 1) * P],
                            wv_t[:, ct, half * 512 : (half + 1) * 512],
                            start=(ct == 0),
                            stop=(ct == CT - 1),
                        )
                    dst = VE[
                        :, tt, half * 8 * DV1 : (half + 1) * 8 * DV1
                    ].rearrange("p (h x) -> p h x", x=DV1)[:, :, 0:D]
                    nc.vector.tensor_add(
                        dst,
                        pv[:].rearrange("p (h d) -> p h d", d=D),
                        bv_bc[:, half * 512 : (half + 1) * 512].rearrange(
                            "p (h d) -> p h d", d=D
                        ),
                    )

                for half in range(2):
                    for tt in range(NT):
                        v_group(half, tt)

                # final-projection weights: load early, transfer overlaps heads
                wp_t = load_w_full(Wp)

                # ============ phase 2: per-head attention + attn_avg ==========
                # attn_avg matmuls are spread one-per-kt-slot through the head
                # loop (16 slots per 2-head cycle = 16 accumulation steps of
                # one attn_avg row-tile), so the PE always has exp-independent
                # filler work and ScalarE stays saturated on exp
                pa = None
                ogs = []

                def normalize_head(hh, og):
                    ct_hh = hh // 2
                    po_hh = (hh % 2) * D
                    rc = misc.tile([1, N], F32, tag="rc")
                    nc.vector.reciprocal(rc[0:1, :], og[D : DV1, :])
                    bc = misc.tile([D, N], F32, tag="bc")
                    nc.gpsimd.partition_broadcast(bc[:], rc[0:1, :], channels=D)
                    nc.gpsimd.tensor_mul(OT[po_hh : po_hh + D, ct_hh, :], og[0:D, :], bc[:])

                def avg_step(qt, a):
                    # one accumulation step of attn_avg row-tile qt
                    nonlocal pa
                    ct_a, nch_a = a // 2, a % 2
                    if a == 0:
                        pa = pap.tile([P, N], F32, tag="pa")
                    nc.tensor.matmul(
                        pa[:, nch_a * 512 : (nch_a + 1) * 512],
                        QT[:, ct_a, qt * P : (qt + 1) * P],
                        KT[:, ct_a, nch_a * 512 : (nch_a + 1) * 512],
                        start=(ct_a == 0),
                        stop=(ct_a == CT - 1),
                    )

                NSLOT = H * NT  # 128 global attention slots

                def qk_slot(g):
                    # scores + exp for global slot g = (head, kt)
                    h, kt = g // NT, g % NT
                    ct_h, po_h = h // 2, (h % 2) * D
                    psum_s = ps.tile([P, N], F32, tag="ps")
                    for nch in range(NCH):
                        nc.tensor.matmul(
                            psum_s[:, nch * 512 : (nch + 1) * 512],
                            KT[po_h : po_h + D, ct_h, kt * P : (kt + 1) * P],
                            QT[po_h : po_h + D, ct_h, nch * 512 : (nch + 1) * 512],
                            start=True,
                            stop=True,
                        )
                    pt = pTp.tile([P, N], F16, tag="pT")
                    nc.scalar.activation(pt[:], psum_s[:], AF.Exp)
                    return pt

                # software pipeline: QK(g+1) issues at slot g, PV(g-1) runs at
                # slot g.  PV lags its exp by a full slot, so the exp->PV
                # semaphore latency is never exposed, and the og staging copy
                # has a whole slot of slack before the next head's first PV
                pts = {0: qk_slot(0)}
                psum_o = None
                for g in range(NSLOT + 1):
                    if g + 1 < NSLOT:
                        pts[g + 1] = qk_slot(g + 1)
                    gp = g - 1  # the slot whose PV work we do now
                    if gp < 0:
                        continue
                    h, kt = gp // NT, gp % NT
                    ct_h, po_h = h // 2, (h % 2) * D
                    qt = h // 2  # attn_avg tile accumulated during this cycle
                    if kt == 0:
                        # normalize the head-before-last here: the chain
                        # (recip -> Pool broadcast -> Pool mul) overlaps this
                        # head's compute and never gates PSUM reuse
                        if ogs and ogs[0][0] < h:
                            normalize_head(*ogs.pop(0))
                        psum_o = po.tile([DV1, N], F32, tag="po")
                    pt = pts.pop(gp)
                    for nch in range(NCH):
                        nc.tensor.matmul(
                            psum_o[:, nch * 512 : (nch + 1) * 512],
                            VE[:, kt, h * DV1 : (h + 1) * DV1],
                            pt[:, nch * 512 : (nch + 1) * 512],
                            start=(kt == 0),
                            stop=(kt == NT - 1),
                        )
                    # one attn_avg step per slot, shifted one slot late so the
                    # previous cycle's epilogue read never blocks the next
                    # cycle's first accumulation
                    t = gp % (2 * NT)  # 0..15 within the 2-head cycle
                    if t >= 1:
                        avg_step(qt, t - 1)
                    if kt < NT - 1:
                        continue
                    # ---- end of head h ----
                    if h < H - 1:
                        # stage PV output out of PSUM immediately so the next
                        # head's accumulation isn't gated on the normalize
                        # chain; split in halves so the first half of the PSUM
                        # bank frees one copy earlier
                        og = ogp.tile([DV1, N], F32, tag="og")
                        nc.vector.tensor_copy(og[:], psum_o[:])
                        ogs.append((h, og))
                    else:
                        # last head: normalize straight from PSUM, split in
                        # halves, so phase 3's ct=7 dependency clears early
                        rc = misc.tile([1, N], F32, tag="rc")
                        nc.vector.reciprocal(rc[0:1, :], psum_o[D : DV1, :])
                        bc = misc.tile([D, N], F32, tag="bc")
                        for hf in range(2):
                            sl = slice(hf * 512, (hf + 1) * 512)
                            nc.gpsimd.partition_broadcast(bc[:, sl], rc[0:1, sl], channels=D)
                            nc.vector.tensor_mul(
                                OT[po_h : po_h + D, ct_h, sl], psum_o[0:D, sl], bc[:, sl]
                            )
                    if h % 2 == 1:
                        avg_step(qt, 15)
                        av = outsb.tile([P, N], F16, tag="o")
                        nc.vector.tensor_scalar_mul(av[:], pa[:], 1.0 / H)
                        nc.sync.dma_start(out=attn_avg[qt * P : (qt + 1) * P, :], in_=av[:])

                # ================= phase 3: final projection ==================
                for qt in range(NT):
                    ot = outsb.tile([P, C], F16, tag="o")
                    if qt < NT - 1:
                        pf = ps.tile([P, N], F32, tag="ps")
                        for ct in range(CT):
                            for nch in range(NCH):
                                nc.tensor.matmul(
                                    pf[:, nch * 512 : (nch + 1) * 512],
                                    OT[:, ct, qt * P : (qt + 1) * P],
                                    wp_t[:, ct, nch * 512 : (nch + 1) * 512],
                                    start=(ct == 0),
                                    stop=(ct == CT - 1),
                                )
                        nc.vector.tensor_add(ot[:], pf[:], bp_bc[:])
                        nc.sync.dma_start(out=out[qt * P : (qt + 1) * P, :], in_=ot[:])
                    else:
                        # last tile: progressively finer column groups with
                        # separate psum tiles, so each group's epilogue + DMA
                        # overlaps the next group's matmuls and the very last
                        # chain is as short as possible
                        for lo, hi in ((0, 512), (512, 768), (768, 1024)):
                            sl = slice(lo, hi)
                            pfh = ps.tile([P, hi - lo], F32, tag="ps")
                            for ct in range(CT):
                                nc.tensor.matmul(
                                    pfh[:],
                                    OT[:, ct, qt * P : (qt + 1) * P],
                                    wp_t[:, ct, sl],
                                    start=(ct == 0),
                                    stop=(ct == CT - 1),
                                )
                            nc.vector.tensor_add(ot[:, sl], pfh[:], bp_bc[:, sl])
                            nc.sync.dma_start(
                                out=out[qt * P : (qt + 1) * P, sl], in_=ot[:, sl]
                            )

    return nc


LAST_RESULT = None
_NC_CACHE = {}


def _get_nc(reps=1):
    if reps not in _NC_CACHE:
        nc = bacc.Bacc("TRN2", target_bir_lowering=False, debug=False)
        _emit(nc, reps)
        nc.compile()
        _NC_CACHE[reps] = nc
    return _NC_CACHE[reps]


def kernel(xq, xk, xv, Wq, bq, Wk, bk, Wv, bv, Wp, bp, **_ignored):
    nc = _get_nc()
    f16 = np.float16
    common = {
        "Wq": np.ascontiguousarray((np.asarray(Wq, np.float32) * np.float32(SCALE)).astype(f16)),
        "bq": np.ascontiguousarray(
            (np.asarray(bq, np.float32) * np.float32(SCALE)).reshape(CT, P).T
        ),
        "Wk": np.ascontiguousarray(np.asarray(Wk, np.float32).astype(f16)),
        "bk": np.ascontiguousarray(np.asarray(bk, np.float32).reshape(CT, P).T),
        "Wv": np.ascontiguousarray(np.asarray(Wv, np.float32).astype(f16)),
        "bv": np.ascontiguousarray(np.asarray(bv, np.float32)),
        "Wp": np.ascontiguousarray(np.asarray(Wp, np.float32).astype(f16)),
        "bp": np.ascontiguousarray(np.asarray(bp, np.float32)),
    }
    def prep_x(x):
        # host pre-transpose to the dim-major [2*P, CT*512] device layout:
        # row (half*P + p), col (ct*512 + t) = x[half*512 + t, ct*128 + p]
        full = np.asarray(x).astype(f16).T.reshape(CT, P, N).transpose(1, 0, 2)
        return np.ascontiguousarray(
            np.concatenate(
                [full[:, :, 0:512].reshape(P, -1), full[:, :, 512:].reshape(P, -1)],
                axis=0,
            )
        )

    in_maps = []
    for b in range(B):
        m = dict(common)
        m["xq"] = prep_x(xq[b])
        m["xk"] = prep_x(xk[b])
        m["xv"] = prep_x(xv[b])
        in_maps.append(m)
    res = run_bass_kernel_spmd(nc, in_maps, list(range(B)))
    global LAST_RESULT
    LAST_RESULT = res
    out = np.stack([np.asarray(res.results[b]["out"], np.float32) for b in range(B)])
    attn_avg = np.stack(
        [np.asarray(res.results[b]["attn_avg"], np.float32) for b in range(B)]
    )
    return out, attn_avg


# revision 80
# speedup vs baseline: 1.0009x; 1.0001x over previous
"""Trainium2 Bass kernel for CrossAttention1D.

Strategy: data-parallel over batch B=8 (one batch per NeuronCore).
Per core, everything is fused in one program, all matmuls in f16
(fp32 PSUM accumulation):
  - xq/xk/xv arrive host-pretransposed to dim-major f16 layout, loaded
    with plain contiguous DMAs (no PE transposes, no PSUM round-trip)
  - Q^T/K^T (dim-major) projections with the bias-add fused into the
    mandatory PSUM->SBUF copy on the vector engine
  - V is projected token-major into a padded [tok, H*65] layout whose
    65th column per head is 1.0, so a single M=65 PV matmul produces
    both the attention output and the softmax denominator (no separate
    ones-matmul)
  - attention runs as a flat 128-slot software pipeline: slot g issues
    QK(g+1) scores, then PV(g-1) — PV lags its exp by a full slot so
    the exp->PV semaphore latency is never exposed; attn_avg = (Q K^T)
    * SCALE / H (sum of per-head dot products == full 1024-dim dot
    product) is spread one matmul per slot as exp-independent filler
  - per head: scores computed TRANSPOSED [k_tok, q_tok] so softmax'd
    probabilities feed PV directly; exp on ScalarE from PSUM; PV output
    is staged out of PSUM immediately and normalized one head deferred
    (reciprocal on DVE, broadcast/multiply on the idle Pool engine)
  - final projection from OT (already in lhsT layout)
SCALE is folded into Wq/bq on the host; outputs are f16, upcast on host.
A short burst of throwaway matmuls keeps the PE p-state clock warm
through the startup DMA wait.
"""

import sys

if "/opt/trn_rl_repo" not in sys.path:
    sys.path.insert(0, "/opt/trn_rl_repo")

import numpy as np

import concourse.bacc as bacc
import concourse.mybir as mybir
from concourse.bass_utils import run_bass_kernel_spmd
from concourse.tile import TileContext

F32 = mybir.dt.float32
F16 = mybir.dt.float16
AF = mybir.ActivationFunctionType

N = 1024   # tokens
C = 1024   # model dim
H = 16     # heads
D = 64     # head dim
DV1 = D + 1  # head dim + fused ones column
P = 128    # partitions
NT = N // P   # 8 token tiles
CT = C // P   # 8 contraction tiles
NCH = 2       # 512-wide chunks per 1024
SCALE = D ** -0.5
B = 8


def _emit(nc, reps=1):
    # xq/xk/xv arrive host-pretransposed to dim-major [2*P, CT*512]: row
    # (half*P + p), col (ct*512 + t) = x[half*512 + t, ct*128 + p].  Each
    # half is one fully-contiguous DMA (8KB runs) — cheaper than the xbar
    # DMA-transpose path and one less instruction class on hardware.
    xq = nc.dram_tensor("xq", [2 * P, CT * 512], F16, kind="ExternalInput")
    xk = nc.dram_tensor("xk", [2 * P, CT * 512], F16, kind="ExternalInput")
    xv = nc.dram_tensor("xv", [2 * P, CT * 512], F16, kind="ExternalInput")
    Wq = nc.dram_tensor("Wq", [C, C], F16, kind="ExternalInput")
    Wk = nc.dram_tensor("Wk", [C, C], F16, kind="ExternalInput")
    Wv = nc.dram_tensor("Wv", [C, C], F16, kind="ExternalInput")
    Wp = nc.dram_tensor("Wp", [C, C], F16, kind="ExternalInput")
    # bq/bk arrive host-pretransposed to [P, CT] (partition-major) so a single
    # contiguous DMA lands them in the per-partition layout
    bq = nc.dram_tensor("bq", [P, CT], F32, kind="ExternalInput")
    bk = nc.dram_tensor("bk", [P, CT], F32, kind="ExternalInput")
    bv = nc.dram_tensor("bv", [C], F32, kind="ExternalInput")
    bp = nc.dram_tensor("bp", [C], F32, kind="ExternalInput")
    out = nc.dram_tensor("out", [N, C], F16, kind="ExternalOutput")
    attn_avg = nc.dram_tensor("attn_avg", [N, N], F16, kind="ExternalOutput")

    with TileContext(nc) as tc:
      for _rep in range(reps):
        with (
            tc.tile_pool(name="persist", bufs=1) as persist,
            tc.tile_pool(name="ps", bufs=2, space="PSUM") as ps,
            tc.tile_pool(name="pa", bufs=1, space="PSUM") as pap,
            tc.tile_pool(name="po", bufs=1, space="PSUM") as po,
        ):
            bq_t = persist.tile([P, CT], F32, tag="bq")
            bk_t = persist.tile([P, CT], F32, tag="bk")
            bv_bc = persist.tile([P, C], F32, tag="bv")
            bp_bc = persist.tile([P, C], F32, tag="bp")

            # persistent activations
            QT = persist.tile([P, CT, N], F16, tag="QT")      # Q^T: [dim, tok]
            KT = persist.tile([P, CT, N], F16, tag="KT")      # K^T: [dim, tok]
            VE = persist.tile([P, NT, H * DV1], F16, tag="VE")  # V: [tok, H*(D+1)]
            OT = persist.tile([P, CT, N], F16, tag="OT")      # out^T: [dim, tok]

            # ones column per head in VE (feeds the fused denominator row)
            for h in range(H):
                nc.vector.memset(VE[:, :, h * DV1 + D : (h + 1) * DV1], 1.0)

            # PE p-state warm-up: the tensor engine clock ramps only after
            # ~3us of continuous work, and resets on idle.  Burn the startup
            # DMA wait with throwaway matmuls on zeroed tiles so the real
            # pipeline starts at full clock.
            zsa = persist.tile([P, P], F16, tag="zsa")
            zsb = persist.tile([P, 512], F16, tag="zsb")
            nc.vector.memset(zsa[:], 0.0)
            nc.vector.memset(zsb[:], 0.0)
            warm_ps = pap.tile([P, N], F32, tag="pa")
            for _ in range(12):
                nc.tensor.matmul(warm_ps[:, 0:512], zsa[:], zsb[:], start=True, stop=True)

            # ================= phase 1: transposed loads + projections ========
            with (
                tc.tile_pool(name="xT", bufs=4) as xTp,
                tc.tile_pool(name="wfull", bufs=3) as wfullp,
                tc.tile_pool(name="outsb", bufs=4) as outsb,
                tc.tile_pool(name="pT", bufs=5) as pTp,
                tc.tile_pool(name="misc", bufs=2) as misc,
                tc.tile_pool(name="ogp", bufs=2) as ogp,
            ):
                def transpose_half(x_dram, half):
                    # separate tile per half => precise dependencies, so the
                    # first projection group starts as soon as half 0 lands
                    xTh = xTp.tile([P, CT, 512], F16, tag="xT")
                    nc.sync.dma_start(
                        out=xTh[:],
                        in_=x_dram[half * P : (half + 1) * P, :].rearrange(
                            "p (ct t) -> p ct t", t=512
                        ),
                    )
                    return xTh

                def load_w_full(W_dram):
                    # whole weight in one DMA: [P, ct, m] layout, 2KB runs
                    w_t = wfullp.tile([P, CT, C], F16, tag="w")
                    nc.sync.dma_start(
                        out=w_t[:], in_=W_dram[:, :].rearrange("(ct p) m -> p ct m", p=P)
                    )
                    return w_t

                def proj_dim_major(lhs_sel, xTh, bias_t, dst):
                    # dst[:, mt, :] = (W^T x^T)[dim-tile mt, tok] + bias
                    for mt in range(CT):
                        pp = ps.tile([P, N], F32, tag="ps")
                        for nch in range(NCH):
                            for ct in range(CT):
                                nc.tensor.matmul(
                                    pp[:, nch * 512 : (nch + 1) * 512],
                                    lhs_sel(mt, ct),
                                    xTh[nch][:, ct, :],
                                    start=(ct == 0),
                                    stop=(ct == CT - 1),
                                )
                        nc.vector.tensor_scalar_add(
                            dst[:, mt, :], pp[:], bias_t[:, mt : mt + 1]
                        )

                # startup order: Q weights (first half, own tile for a precise
                # dependency), xq half 0 => first matmul group fires as soon
                # as both land; rest pipelines
                def load_wq_half(hf):
                    wqh = wfullp.tile([P, CT, 512], F16, tag="w")
                    nc.sync.dma_start(
                        out=wqh[:],
                        in_=Wq[:, hf * 512 : (hf + 1) * 512].rearrange(
                            "(ct p) m -> p ct m", p=P
                        ),
                    )
                    return wqh

                wq_h = [load_wq_half(0)]
                xq0 = transpose_half(xq, 0)
                xq1 = transpose_half(xq, 1)
                wq_h.append(load_wq_half(1))
                # biases behind the startup transposes in the SP FIFO, so they
                # can't be hoisted ahead of them by an idle sequencer
                nc.sync.dma_start(out=bq_t[:], in_=bq[:, :])
                nc.sync.dma_start(out=bk_t[:], in_=bk[:, :])
                nc.sync.dma_start(out=bv_bc[0:1, :], in_=bv[None, :])
                nc.sync.dma_start(out=bp_bc[0:1, :], in_=bp[None, :])
                nc.gpsimd.partition_broadcast(bv_bc[:], bv_bc[0:1, :], channels=P)
                nc.gpsimd.partition_broadcast(bp_bc[:], bp_bc[0:1, :], channels=P)
                proj_dim_major(
                    lambda mt, ct: wq_h[mt // 4][:, ct, (mt % 4) * P : (mt % 4 + 1) * P],
                    [xq0, xq1],
                    bq_t,
                    QT,
                )

                xk0 = transpose_half(xk, 0)
                wk_t = load_w_full(Wk)
                xk1 = transpose_half(xk, 1)
                proj_dim_major(
                    lambda mt, ct: wk_t[:, ct, mt * P : (mt + 1) * P],
                    [xk0, xk1],
                    bk_t,
                    KT,
                )

                xv0 = transpose_half(xv, 0)
                xv1 = transpose_half(xv, 1)
                xvT = [xv0, xv1]
                wv_t = load_w_full(Wv)

                # V token-major into padded VE: VE[tt, h*65:(h*65+64)] =
                # xv @ Wv + bv
                def v_group(half, tt):
                    pv = ps.tile([P, 512], F32, tag="ps")
                    for ct in range(CT):
                        nc.tensor.matmul(
                            pv[:],
                            xvT[tt // 4][:, ct, (tt % 4) * P : (tt % 4 + 1) * P],
                            wv_t[:, ct, half * 512 : (half + 1) * 512],
                            start=(ct == 0),
                            stop=(ct == CT - 1),
                        )
                    dst = VE[
                        :, tt, half * 8 * DV1 : (half + 1) * 8 * DV1
                    ].rearrange("p (h x) -> p h x", x=DV1)[:, :, 0:D]
                    nc.vector.tensor_add(
                        dst,
                        pv[:].rearrange("p (h d) -> p h d", d=D),
                        bv_bc[:, half * 512 : (half + 1) * 512].rearrange(
                            "p (h d) -> p h d", d=D
                        ),
                    )

                for half in range(2):
                    for tt in range(NT):
                        v_group(half, tt)

                # final-projection weights: load early, transfer overlaps heads
                wp_t = load_w_full(Wp)

                # ============ phase 2: per-head attention + attn_avg ==========
                # attn_avg matmuls are spread one-per-kt-slot through the head
                # loop (16 slots per 2-head cycle = 16 accumulation steps of
                # one attn_avg row-tile), so the PE always has exp-independent
                # filler work and ScalarE stays saturated on exp
                pa = None
                ogs = []

                def normalize_head(hh, og):
                    ct_hh = hh // 2
                    po_hh = (hh % 2) * D
                    rc = misc.tile([1, N], F32, tag="rc")
                    nc.vector.reciprocal(rc[0:1, :], og[D : DV1, :])
                    bc = misc.tile([D, N], F32, tag="bc")
                    nc.gpsimd.partition_broadcast(bc[:], rc[0:1, :], channels=D)
                    nc.gpsimd.tensor_mul(OT[po_hh : po_hh + D, ct_hh, :], og[0:D, :], bc[:])

                def avg_step(qt, a):
                    # one accumulation step of attn_avg row-tile qt
                    nonlocal pa
                    ct_a, nch_a = a // 2, a % 2
                    if a == 0:
                        pa = pap.tile([P, N], F32, tag="pa")
                    nc.tensor.matmul(
                        pa[:, nch_a * 512 : (nch_a + 1) * 512],
                        QT[:, ct_a, qt * P : (qt + 1) * P],
                        KT[:, ct_a, nch_a * 512 : (nch_a + 1) * 512],
                        start=(ct_a == 0),
                        stop=(ct_a == CT - 1),
                    )

                NSLOT = H * NT  # 128 global attention slots

                def qk_slot(g):
                    # scores + exp for global slot g = (head, kt)
                    h, kt = g // NT, g % NT
                    ct_h, po_h = h // 2, (h % 2) * D
                    psum_s = ps.tile([P, N], F32, tag="ps")
                    for nch in range(NCH):
                        nc.tensor.matmul(
                            psum_s[:, nch * 512 : (nch + 1) * 512],
                            KT[po_h : po_h + D, ct_h, kt * P : (kt + 1) * P],
                            QT[po_h : po_h + D, ct_h, nch * 512 : (nch + 1) * 512],
                            start=True,
                            stop=True,
                        )
                    pt = pTp.tile([P, N], F16, tag="pT")
                    nc.scalar.activation(pt[:], psum_s[:], AF.Exp)
                    return pt

                # software pipeline: QK(g+1) issues at slot g, PV(g-1) runs at
                # slot g.  PV lags its exp by a full slot, so the exp->PV
                # semaphore latency is never exposed, and the og staging copy
                # has a whole slot of slack before the next head's first PV
                pts = {0: qk_slot(0)}
                psum_o = None
                for g in range(NSLOT + 1):
                    if g + 1 < NSLOT:
                        pts[g + 1] = qk_slot(g + 1)
                    gp = g - 1  # the slot whose PV work we do now
                    if gp < 0:
                        continue
                    h, kt = gp // NT, gp % NT
                    ct_h, po_h = h // 2, (h % 2) * D
                    qt = h // 2  # attn_avg tile accumulated during this cycle
                    if kt == 0:
                        # normalize the head-before-last here: the chain
                        # (recip -> Pool broadcast -> Pool mul) overlaps this
                        # head's compute and never gates PSUM reuse
                        if ogs and ogs[0][0] < h:
                            normalize_head(*ogs.pop(0))
                        psum_o = po.tile([DV1, N], F32, tag="po")
                    pt = pts.pop(gp)
                    for nch in range(NCH):
                        nc.tensor.matmul(
                            psum_o[:, nch * 512 : (nch + 1) * 512],
                            VE[:, kt, h * DV1 : (h + 1) * DV1],
                            pt[:, nch * 512 : (nch + 1) * 512],
                            start=(kt == 0),
                            stop=(kt == NT - 1),
                        )
                    # one attn_avg step per slot, shifted one slot late so the
                    # previous cycle's epilogue read never blocks the next
                    # cycle's first accumulation
                    t = gp % (2 * NT)  # 0..15 within the 2-head cycle
                    if t >= 1:
                        avg_step(qt, t - 1)
                    if kt < NT - 1:
                        continue
                    # ---- end of head h ----
                    if h < H - 1:
                        # stage PV output out of PSUM immediately so the next
                        # head's accumulation isn't gated on the normalize
                        # chain; split in halves so the first half of the PSUM
                        # bank frees one copy earlier
                        og = ogp.tile([DV1, N], F32, tag="og")
                        nc.vector.tensor_copy(og[:], psum_o[:])
                        ogs.append((h, og))
                    else:
                        # last head: normalize straight from PSUM, split in
                        # halves, so phase 3's ct=7 dependency clears early
                        rc = misc.tile([1, N], F32, tag="rc")
                        nc.vector.reciprocal(rc[0:1, :], psum_o[D : DV1, :])
                        bc = misc.tile([D, N], F32, tag="bc")
                        for hf in range(2):
                            sl = slice(hf * 512, (hf + 1) * 512)
                            nc.gpsimd.partition_broadcast(bc[:, sl], rc[0:1, sl], channels=D)
                            nc.vector.tensor_mul(
                                OT[po_h : po_h + D, ct_h, sl], psum_o[0:D, sl], bc[:, sl]
                            )
                    if h % 2 == 1:
                        avg_step(qt, 15)
                        av = outsb.tile([P, N], F16, tag="o")
                        nc.vector.tensor_scalar_mul(av[:], pa[:], 1.0 / H)
                        nc.sync.dma_start(out=attn_avg[qt * P : (qt + 1) * P, :], in_=av[:])

                # ================= phase 3: final projection ==================
                for qt in range(NT):
                    ot = outsb.tile([P, C], F16, tag="o")
                    if qt < NT - 1:
                        pf = ps.tile([P, N], F32, tag="ps")
                        for ct in range(CT):
                            for nch in range(NCH):
                                nc.tensor.matmul(
                                    pf[:, nch * 512 : (nch + 1) * 512],
                                    OT[:, ct, qt * P : (qt + 1) * P],
                                    wp_t[:, ct, nch * 512 : (nch + 1) * 512],
                                    start=(ct == 0),
                                    stop=(ct == CT - 1),
                                )
                        nc.vector.tensor_add(ot[:], pf[:], bp_bc[:])
                        nc.sync.dma_start(out=out[qt * P : (qt + 1) * P, :], in_=ot[:])
                    else:
                        # last tile: progressively finer column groups with
                        # separate psum tiles, so each group's epilogue + DMA
                        # overlaps the next group's matmuls and the very last
                        # chain is as short as possible
                        for lo, hi in ((0, 512), (512, 768), (768, 1024)):
                            sl = slice(lo, hi)
                            pfh = ps.tile([P, hi - lo], F32, tag="ps")
                            for ct in range(CT):
                                nc.tensor.matmul(
                                    pfh[:],
                                    OT[:, ct, qt * P : (qt + 1) * P],
                                    wp_t[:, ct, sl],
                                    start=(ct == 0),
                                    stop=(ct == CT - 1),
                                )
                            nc.vector.tensor_add(ot[:, sl], pfh[:], bp_bc[:, sl])
                            nc.sync.dma_start(
                                out=out[qt * P : (qt + 1) * P, sl], in_=ot[:, sl]
                            )

    return nc


LAST_RESULT = None
_NC_CACHE = {}


def _get_nc(reps=1):
    if reps not in _NC_CACHE:
        nc = bacc.Bacc("TRN2", target_bir_lowering=False, debug=False)
        _emit(nc, reps)
        nc.compile()
        _NC_CACHE[reps] = nc
    return _NC_CACHE[reps]


def kernel(xq, xk, xv, Wq, bq, Wk, bk, Wv, bv, Wp, bp, **_ignored):
    nc = _get_nc()
    f16 = np.float16
    common = {
        "Wq": np.ascontiguousarray((np.asarray(Wq, np.float32) * np.float32(SCALE)).astype(f16)),
        "bq": np.ascontiguousarray(
            (np.asarray(bq, np.float32) * np.float32(SCALE)).reshape(CT, P).T
        ),
        "Wk": np.ascontiguousarray(np.asarray(Wk, np.float32).astype(f16)),
        "bk": np.ascontiguousarray(np.asarray(bk, np.float32).reshape(CT, P).T),
        "Wv": np.ascontiguousarray(np.asarray(Wv, np.float32).astype(f16)),
        "bv": np.ascontiguousarray(np.asarray(bv, np.float32)),
        "Wp": np.ascontiguousarray(np.asarray(Wp, np.float32).astype(f16)),
        "bp": np.ascontiguousarray(np.asarray(bp, np.float32)),
    }
    def prep_x(x):
        # host pre-transpose to the dim-major [2*P, CT*512] device layout:
        # row (half*P + p), col (ct*512 + t) = x[half*512 + t, ct*128 + p]
        full = np.asarray(x).astype(f16).T.reshape(CT, P, N).transpose(1, 0, 2)
        return np.ascontiguousarray(
            np.concatenate(
                [full[:, :, 0:512].reshape(P, -1), full[:, :, 512:].reshape(P, -1)],
                axis=0,
            )
        )

    in_maps = []
    for b in range(B):
        m = dict(common)
        m["xq"] = prep_x(xq[b])
        m["xk"] = prep_x(xk[b])
        m["xv"] = prep_x(xv[b])
        in_maps.append(m)
    res = run_bass_kernel_spmd(nc, in_maps, list(range(B)))
    global LAST_RESULT
    LAST_RESULT = res
    out = np.stack([np.asarray(res.results[b]["out"], np.float32) for b in range(B)])
    attn_avg = np.stack(
        [np.asarray(res.results[b]["attn_avg"], np.float32) for b in range(B)]
    )
    return out, attn_avg
